# revision 9
# baseline (speedup 1.0000x reference)
"""Trainium2 Bass kernel for segment_sum (scatter-add of edge features into nodes).

Strategy: 2M edges split contiguously across 8 NeuronCores (250k each).
Host-side prep (layout only, no FP arithmetic): sort each core's edges by
node id, pad every node run to EVEN length (pad slots carry h=0 inside
the run), cut the padded stream into 128 partition streams at run
boundaries, pad each to 2560 slots, and deinterleave each channel row
into step-1 A/B halves (A = even slots, B = odd slots of each pair).

Device (per core):
  1. Pairing pass: A += B with a plain tensor_tensor add (on GPSIMD if
     GPSIMD_TT, else DVE). All operands are bf16 step-1 (DVE would run
     2x mode); each pair is two edges of the SAME node (guaranteed by
     the even-run padding), so A becomes the pair-sum stream at half
     the length.
  2. Segmented scan over A on the DVE:  state = m2[t]*state + A[t]
     (fp32 state, ~2.08 ns/el) per feature channel. At the last pair of
     each node run, state holds that node's complete per-core sum.
Single 2560-slot piece; input DMAs, pairing, scans and output DMAs are
pipelined per 4-channel group. The A halves are DMA'd back (bf16); the
host picks the run-end pairs and adds the 8 per-core partials (same
unshard-add as the original baseline).
"""
import numpy as np
import ml_dtypes

import concourse.bass as bass
import concourse.bacc as bacc
import concourse.mybir as mybir
from concourse import tile
from concourse.bass_utils import run_bass_kernel_spmd

BF16 = mybir.dt.bfloat16
OP = mybir.AluOpType

E = 2_000_000
D = 32
N = 100_000
CORES = 8
EPC = E // CORES            # 250_000
PARTS = 128
SLOTS = 2560                # even-run-padded raw slots per partition
HLEN = SLOTS // 2           # 1280 pairs per channel row
FREE = D * SLOTS
GPSIMD_TT = True            # run the pairing adds on GPSIMD (DVE if False)
IN_GROUPS = [2, 2, 4, 4, 4, 4, 4, 4, 4]
TTG = 4                     # channels per pairing-TT group
# output d-groups (fine tail shrinks the final-DMA drain)
OUT_GROUPS = [4, 4, 4, 4, 4, 4, 4, 2, 1, 1]


def build_program():
    nc = bacc.Bacc("TRN2", target_bir_lowering=False, debug=False,
                   num_devices=CORES)
    h_in = nc.dram_tensor("h", [PARTS, FREE], BF16, kind="ExternalInput")
    m_in = nc.dram_tensor("m", [PARTS, HLEN], BF16, kind="ExternalInput")
    s_out = nc.dram_tensor("s", [PARTS, D * HLEN], BF16,
                           kind="ExternalOutput")
    tt_engine = nc.gpsimd if GPSIMD_TT else nc.vector

    with tile.TileContext(nc) as tc:
        with tc.tile_pool(name="mask", bufs=1) as mp, \
             tc.tile_pool(name="work", bufs=1) as wp:
            mt = mp.tile([PARTS, HLEN], BF16)
            nc.sync.dma_start(mt[:], m_in[:])
            ht = wp.tile([PARTS, FREE], BF16)
            d0 = 0
            for ng in IN_GROUPS:
                nc.sync.dma_start(
                    ht[:, d0 * SLOTS:(d0 + ng) * SLOTS],
                    h_in[:, d0 * SLOTS:(d0 + ng) * SLOTS])
                d0 += ng
            # pairing pass: A[d, j] += B[d, j], in place
            for g in range(D // TTG):
                base = g * TTG * SLOTS
                a_ap = bass.AP(ht.tensor, base,
                               [[FREE, PARTS], [SLOTS, TTG], [1, HLEN]])
                b_ap = bass.AP(ht.tensor, base + HLEN,
                               [[FREE, PARTS], [SLOTS, TTG], [1, HLEN]])
                tt_engine.tensor_tensor(a_ap, a_ap, b_ap, OP.add)
            d = 0
            for ng in OUT_GROUPS:
                for dd in range(d, d + ng):
                    lo = dd * SLOTS
                    # in-place segmented scan over the A half
                    nc.vector.tensor_tensor_scan(
                        ht[:, lo:lo + HLEN], mt[:],
                        ht[:, lo:lo + HLEN], 0.0, OP.mult, OP.add)
                src = bass.AP(ht.tensor, d * SLOTS,
                              [[FREE, PARTS], [SLOTS, ng], [1, HLEN]])
                nc.sync.dma_start(
                    s_out[:, d * HLEN:(d + ng) * HLEN], src)
                d += ng
    nc.compile()
    return nc


_prog_cache = {}


def _get_prog():
    if "nc" not in _prog_cache:
        _prog_cache["nc"] = build_program()
    return _prog_cache["nc"]


def kernel(H, X_node, node_num):
    H = np.ascontiguousarray(np.asarray(H, dtype=np.float32))
    X = np.asarray(X_node).astype(np.int64)
    assert H.shape == (E, D) and X.shape == (E,)
    nc = _get_prog()

    in_maps = []
    metas = []
    for c in range(CORES):
        Xc = X[c * EPC:(c + 1) * EPC]
        Hc = H[c * EPC:(c + 1) * EPC]
        perm = np.argsort(Xc, kind="stable")
        Xs = Xc[perm]
        Hs = Hc[perm]
        # node runs; pad each run to even length (pad slot: h=0, same node)
        runstarts = np.concatenate([[0], np.flatnonzero(np.diff(Xs)) + 1])
        R = len(runstarts)
        L = np.diff(np.concatenate([runstarts, [EPC]]))
        odd = (L & 1).astype(bool)
        start2 = np.concatenate([[0], np.cumsum(L + (L & 1))])
        T = int(start2[-1])
        run_of = np.repeat(np.arange(R), L)
        pos2 = start2[:-1][run_of] + (np.arange(EPC) - runstarts[run_of])
        node2 = np.full(T, -1, np.int64)
        h2 = np.zeros((T, D), np.float32)
        node2[pos2] = Xs
        h2[pos2] = Hs
        node2[start2[1:][odd] - 1] = Xs[runstarts[odd]]

        # cut the padded stream at run boundaries into 128 streams
        tgt = np.arange(1, PARTS) * ((T + PARTS - 1) // PARTS)
        ci = np.minimum(np.searchsorted(start2[:-1], tgt), R - 1)
        cuts = np.concatenate([[0], start2[:-1][ci], [T]])
        cnt = np.diff(cuts)
        assert cnt.max() <= SLOTS, f"partition stream overflow: {cnt.max()}"

        node_pad = np.full((PARTS, SLOTS), -1, np.int64)
        h_pad = np.zeros((PARTS, SLOTS, D), np.float32)
        pidx = np.repeat(np.arange(PARTS), cnt)
        eidx = np.arange(T) - np.repeat(cuts[:-1], cnt)
        node_pad[pidx, eidx] = node2
        h_pad[pidx, eidx] = h2
        m = np.zeros((PARTS, SLOTS), np.float32)
        m[:, 1:] = node_pad[:, 1:] == node_pad[:, :-1]
        m2 = m[:, 0::2]                       # pair-level mask [PARTS, 1280]

        # device layout per channel row: [A(1280) | B(1280)]
        dev = np.empty((PARTS, D, SLOTS), np.float32)
        dev[:, :, :HLEN] = h_pad[:, 0::2, :].transpose(0, 2, 1)
        dev[:, :, HLEN:] = h_pad[:, 1::2, :].transpose(0, 2, 1)
        h_dev = dev.reshape(PARTS, FREE).astype(ml_dtypes.bfloat16)
        m_dev = np.ascontiguousarray(m2).astype(ml_dtypes.bfloat16)
        in_maps.append({"h": np.ascontiguousarray(h_dev), "m": m_dev})
        metas.append(node_pad[:, 0::2])       # node id per pair

    _prog_cache["last_inputs"] = in_maps
    # The very first execution of a freshly loaded program has been
    # observed (once) to return corrupted results; correct runs are
    # bit-identical. Run until two consecutive executions agree.
    res = run_bass_kernel_spmd(nc, in_maps, core_ids=list(range(CORES)),
                               trace=False)
    for _ in range(3):
        res2 = run_bass_kernel_spmd(nc, in_maps, core_ids=list(range(CORES)),
                                    trace=False)
        if all(
            np.array_equal(
                res.results[c]["s"].view(np.uint16),
                res2.results[c]["s"].view(np.uint16))
            for c in range(CORES)
        ):
            break
        res = res2

    out = np.zeros((N, D), np.float32)
    for c in range(CORES):
        node_pair = metas[c]                  # [PARTS, 1280]
        s = np.asarray(res.results[c]["s"]).astype(np.float32)
        s = s.reshape(PARTS, D, HLEN)
        nxt = np.concatenate(
            [node_pair[:, 1:], np.full((PARTS, 1), -2, np.int64)], axis=1)
        is_end = (node_pair >= 0) & (node_pair != nxt)
        pp, ii = np.nonzero(is_end)
        nodes = node_pair[pp, ii]
        vals = s[pp, :, ii]
        # within one core each node has exactly one run end -> unique idx
        out[nodes] += vals
    return out


# revision 10
# speedup vs baseline: 1.4283x; 1.4283x over previous
"""Trainium2 Bass kernel for segment_sum (scatter-add of edge features into nodes).

Strategy: 2M edges split contiguously across 8 NeuronCores (250k each).
Host-side prep (layout only, no FP arithmetic): sort each core's edges by
node id, pad every node run to EVEN length (pad slots carry h=0 inside
the run), cut the padded stream into 128 partition streams at run
boundaries, pad each to 2560 slots, and deinterleave each channel row
into step-1 A/B halves (A = even slots, B = odd slots of each pair).

Device (per core):
  1. Pairing pass: A += B with a plain tensor_tensor add (on GPSIMD if
     GPSIMD_TT, else DVE). All operands are bf16 step-1 (DVE would run
     2x mode); each pair is two edges of the SAME node (guaranteed by
     the even-run padding), so A becomes the pair-sum stream at half
     the length.
  2. Segmented scan over A on the DVE:  state = m2[t]*state + A[t]
     (fp32 state, ~2.08 ns/el) per feature channel. At the last pair of
     each node run, state holds that node's complete per-core sum.
Single 2560-slot piece; input DMAs, pairing, scans and output DMAs are
pipelined per 4-channel group. The A halves are DMA'd back (bf16); the
host picks the run-end pairs and adds the 8 per-core partials (same
unshard-add as the original baseline).
"""
import numpy as np
import ml_dtypes

import concourse.bass as bass
import concourse.bacc as bacc
import concourse.mybir as mybir
from concourse import tile
from concourse.bass_utils import run_bass_kernel_spmd

BF16 = mybir.dt.bfloat16
OP = mybir.AluOpType

E = 2_000_000
D = 32
N = 100_000
CORES = 8
EPC = E // CORES            # 250_000
PARTS = 128
SLOTS = 2560                # even-run-padded raw slots per partition
HLEN = SLOTS // 2           # 1280 pairs per channel row
FREE = D * SLOTS
GPSIMD_TT = False           # run the pairing adds on GPSIMD (DVE if False)
IN_GROUPS = [2, 2, 4, 4, 4, 4, 4, 4, 4]
TTG = 4                     # channels per pairing-TT group
# output d-groups (fine tail shrinks the final-DMA drain)
OUT_GROUPS = [4, 4, 4, 4, 4, 4, 4, 2, 1, 1]


def build_program():
    nc = bacc.Bacc("TRN2", target_bir_lowering=False, debug=False,
                   num_devices=CORES)
    h_in = nc.dram_tensor("h", [PARTS, FREE], BF16, kind="ExternalInput")
    m_in = nc.dram_tensor("m", [PARTS, HLEN], BF16, kind="ExternalInput")
    s_out = nc.dram_tensor("s", [PARTS, D * HLEN], BF16,
                           kind="ExternalOutput")
    tt_engine = nc.gpsimd if GPSIMD_TT else nc.vector

    with tile.TileContext(nc) as tc:
        with tc.tile_pool(name="mask", bufs=1) as mp, \
             tc.tile_pool(name="work", bufs=1) as wp:
            mt = mp.tile([PARTS, HLEN], BF16)
            nc.sync.dma_start(mt[:], m_in[:])
            ht = wp.tile([PARTS, FREE], BF16)
            d0 = 0
            for ng in IN_GROUPS:
                nc.sync.dma_start(
                    ht[:, d0 * SLOTS:(d0 + ng) * SLOTS],
                    h_in[:, d0 * SLOTS:(d0 + ng) * SLOTS])
                d0 += ng
            # pairing pass: A[d, j] += B[d, j], in place
            for g in range(D // TTG):
                base = g * TTG * SLOTS
                a_ap = bass.AP(ht.tensor, base,
                               [[FREE, PARTS], [SLOTS, TTG], [1, HLEN]])
                b_ap = bass.AP(ht.tensor, base + HLEN,
                               [[FREE, PARTS], [SLOTS, TTG], [1, HLEN]])
                tt_engine.tensor_tensor(a_ap, a_ap, b_ap, OP.add)
            d = 0
            for ng in OUT_GROUPS:
                for dd in range(d, d + ng):
                    lo = dd * SLOTS
                    # in-place segmented scan over the A half
                    nc.vector.tensor_tensor_scan(
                        ht[:, lo:lo + HLEN], mt[:],
                        ht[:, lo:lo + HLEN], 0.0, OP.mult, OP.add)
                src = bass.AP(ht.tensor, d * SLOTS,
                              [[FREE, PARTS], [SLOTS, ng], [1, HLEN]])
                nc.sync.dma_start(
                    s_out[:, d * HLEN:(d + ng) * HLEN], src)
                d += ng
    nc.compile()
    return nc


_prog_cache = {}


def _get_prog():
    if "nc" not in _prog_cache:
        _prog_cache["nc"] = build_program()
    return _prog_cache["nc"]


def kernel(H, X_node, node_num):
    H = np.ascontiguousarray(np.asarray(H, dtype=np.float32))
    X = np.asarray(X_node).astype(np.int64)
    assert H.shape == (E, D) and X.shape == (E,)
    nc = _get_prog()

    in_maps = []
    metas = []
    for c in range(CORES):
        Xc = X[c * EPC:(c + 1) * EPC]
        Hc = H[c * EPC:(c + 1) * EPC]
        perm = np.argsort(Xc, kind="stable")
        Xs = Xc[perm]
        Hs = Hc[perm]
        # node runs; pad each run to even length (pad slot: h=0, same node)
        runstarts = np.concatenate([[0], np.flatnonzero(np.diff(Xs)) + 1])
        R = len(runstarts)
        L = np.diff(np.concatenate([runstarts, [EPC]]))
        odd = (L & 1).astype(bool)
        start2 = np.concatenate([[0], np.cumsum(L + (L & 1))])
        T = int(start2[-1])
        run_of = np.repeat(np.arange(R), L)
        pos2 = start2[:-1][run_of] + (np.arange(EPC) - runstarts[run_of])
        node2 = np.full(T, -1, np.int64)
        h2 = np.zeros((T, D), np.float32)
        node2[pos2] = Xs
        h2[pos2] = Hs
        node2[start2[1:][odd] - 1] = Xs[runstarts[odd]]

        # cut the padded stream at run boundaries into 128 streams
        tgt = np.arange(1, PARTS) * ((T + PARTS - 1) // PARTS)
        ci = np.minimum(np.searchsorted(start2[:-1], tgt), R - 1)
        cuts = np.concatenate([[0], start2[:-1][ci], [T]])
        cnt = np.diff(cuts)
        assert cnt.max() <= SLOTS, f"partition stream overflow: {cnt.max()}"

        node_pad = np.full((PARTS, SLOTS), -1, np.int64)
        h_pad = np.zeros((PARTS, SLOTS, D), np.float32)
        pidx = np.repeat(np.arange(PARTS), cnt)
        eidx = np.arange(T) - np.repeat(cuts[:-1], cnt)
        node_pad[pidx, eidx] = node2
        h_pad[pidx, eidx] = h2
        m = np.zeros((PARTS, SLOTS), np.float32)
        m[:, 1:] = node_pad[:, 1:] == node_pad[:, :-1]
        m2 = m[:, 0::2]                       # pair-level mask [PARTS, 1280]

        # device layout per channel row: [A(1280) | B(1280)]
        dev = np.empty((PARTS, D, SLOTS), np.float32)
        dev[:, :, :HLEN] = h_pad[:, 0::2, :].transpose(0, 2, 1)
        dev[:, :, HLEN:] = h_pad[:, 1::2, :].transpose(0, 2, 1)
        h_dev = dev.reshape(PARTS, FREE).astype(ml_dtypes.bfloat16)
        m_dev = np.ascontiguousarray(m2).astype(ml_dtypes.bfloat16)
        in_maps.append({"h": np.ascontiguousarray(h_dev), "m": m_dev})
        metas.append(node_pad[:, 0::2])       # node id per pair

    _prog_cache["last_inputs"] = in_maps
    # The very first execution of a freshly loaded program has been
    # observed (once) to return corrupted results; correct runs are
    # bit-identical. Run until two consecutive executions agree.
    res = run_bass_kernel_spmd(nc, in_maps, core_ids=list(range(CORES)),
                               trace=False)
    for _ in range(3):
        res2 = run_bass_kernel_spmd(nc, in_maps, core_ids=list(range(CORES)),
                                    trace=False)
        if all(
            np.array_equal(
                res.results[c]["s"].view(np.uint16),
                res2.results[c]["s"].view(np.uint16))
            for c in range(CORES)
        ):
            break
        res = res2

    out = np.zeros((N, D), np.float32)
    for c in range(CORES):
        node_pair = metas[c]                  # [PARTS, 1280]
        s = np.asarray(res.results[c]["s"]).astype(np.float32)
        s = s.reshape(PARTS, D, HLEN)
        nxt = np.concatenate(
            [node_pair[:, 1:], np.full((PARTS, 1), -2, np.int64)], axis=1)
        is_end = (node_pair >= 0) & (node_pair != nxt)
        pp, ii = np.nonzero(is_end)
        nodes = node_pair[pp, ii]
        vals = s[pp, :, ii]
        # within one core each node has exactly one run end -> unique idx
        out[nodes] += vals
    return out


# revision 12
# speedup vs baseline: 1.7295x; 1.2109x over previous
"""Trainium2 Bass kernel for segment_sum (scatter-add of edge features into nodes).

Strategy: 2M edges split contiguously across 8 NeuronCores (250k each).
Host-side prep (layout only, no FP arithmetic): sort each core's edges by
node id, pad every node run to EVEN length (pad slots carry h=0 inside
the run), cut the padded stream into 128 partition streams at run
boundaries, pad each to 2560 slots, and deinterleave each channel row
into step-1 A/B halves (A = even slots, B = odd slots of each pair).

Device (per core):
  1. Pairing pass: A += B with a plain tensor_tensor add (on GPSIMD if
     GPSIMD_TT, else DVE). All operands are bf16 step-1 (DVE would run
     2x mode); each pair is two edges of the SAME node (guaranteed by
     the even-run padding), so A becomes the pair-sum stream at half
     the length.
  2. Segmented scan over A on the DVE:  state = m2[t]*state + A[t]
     (fp32 state, ~2.08 ns/el) per feature channel. At the last pair of
     each node run, state holds that node's complete per-core sum.
Single 2560-slot piece; input DMAs, pairing, scans and output DMAs are
pipelined per 4-channel group. The A halves are DMA'd back (bf16); the
host picks the run-end pairs and adds the 8 per-core partials (same
unshard-add as the original baseline).
"""
import numpy as np
import ml_dtypes

import concourse.bass as bass
import concourse.bacc as bacc
import concourse.mybir as mybir
from concourse import tile
from concourse.bass_utils import run_bass_kernel_spmd

BF16 = mybir.dt.bfloat16
OP = mybir.AluOpType

E = 2_000_000
D = 32
N = 100_000
CORES = 8
EPC = E // CORES            # 250_000
PARTS = 128
SLOTS = 2560                # even-run-padded raw slots per partition
HLEN = SLOTS // 2           # 1280 pairs per channel row
FREE = D * SLOTS
GPSIMD_TT = False           # run the pairing adds on GPSIMD (DVE if False)
# channel groups: DMA-in, pairing-TT, scans and DMA-out are pipelined
# per group; the first groups are small so compute starts early, and
# the final out-DMAs are split fine to shrink the tail
GROUPS = [2, 2, 4, 4, 4, 4, 4, 4, 4]
OUT_SPLIT = {8: [2, 1, 1]}  # group idx -> out-DMA sub-splits


def build_program():
    nc = bacc.Bacc("TRN2", target_bir_lowering=False, debug=False,
                   num_devices=CORES)
    h_in = nc.dram_tensor("h", [PARTS, FREE], BF16, kind="ExternalInput")
    m_in = nc.dram_tensor("m", [PARTS, HLEN], BF16, kind="ExternalInput")
    s_out = nc.dram_tensor("s", [PARTS, D * HLEN], BF16,
                           kind="ExternalOutput")
    tt_engine = nc.gpsimd if GPSIMD_TT else nc.vector

    with tile.TileContext(nc) as tc:
        with tc.tile_pool(name="mask", bufs=1) as mp, \
             tc.tile_pool(name="work", bufs=1) as wp:
            mt = mp.tile([PARTS, HLEN], BF16)
            nc.sync.dma_start(mt[:], m_in[:])
            ht = wp.tile([PARTS, FREE], BF16)
            d0 = 0
            for ng in GROUPS:
                nc.sync.dma_start(
                    ht[:, d0 * SLOTS:(d0 + ng) * SLOTS],
                    h_in[:, d0 * SLOTS:(d0 + ng) * SLOTS])
                d0 += ng
            d = 0
            for gi, ng in enumerate(GROUPS):
                # pairing pass: A[d, j] += B[d, j], in place
                base = d * SLOTS
                a_ap = bass.AP(ht.tensor, base,
                               [[FREE, PARTS], [SLOTS, ng], [1, HLEN]])
                b_ap = bass.AP(ht.tensor, base + HLEN,
                               [[FREE, PARTS], [SLOTS, ng], [1, HLEN]])
                tt_engine.tensor_tensor(a_ap, a_ap, b_ap, OP.add)
                for dd in range(d, d + ng):
                    lo = dd * SLOTS
                    # in-place segmented scan over the A half
                    nc.vector.tensor_tensor_scan(
                        ht[:, lo:lo + HLEN], mt[:],
                        ht[:, lo:lo + HLEN], 0.0, OP.mult, OP.add)
                for so, sn in zip(
                        np.cumsum([0] + OUT_SPLIT.get(gi, [ng]))[:-1],
                        OUT_SPLIT.get(gi, [ng])):
                    src = bass.AP(ht.tensor, (d + int(so)) * SLOTS,
                                  [[FREE, PARTS], [SLOTS, int(sn)],
                                   [1, HLEN]])
                    nc.sync.dma_start(
                        s_out[:, (d + int(so)) * HLEN:
                              (d + int(so) + int(sn)) * HLEN], src)
                d += ng
    nc.compile()
    return nc


_prog_cache = {}


def _get_prog():
    if "nc" not in _prog_cache:
        _prog_cache["nc"] = build_program()
    return _prog_cache["nc"]


def kernel(H, X_node, node_num):
    H = np.ascontiguousarray(np.asarray(H, dtype=np.float32))
    X = np.asarray(X_node).astype(np.int64)
    assert H.shape == (E, D) and X.shape == (E,)
    nc = _get_prog()

    in_maps = []
    metas = []
    for c in range(CORES):
        Xc = X[c * EPC:(c + 1) * EPC]
        Hc = H[c * EPC:(c + 1) * EPC]
        perm = np.argsort(Xc, kind="stable")
        Xs = Xc[perm]
        Hs = Hc[perm]
        # node runs; pad each run to even length (pad slot: h=0, same node)
        runstarts = np.concatenate([[0], np.flatnonzero(np.diff(Xs)) + 1])
        R = len(runstarts)
        L = np.diff(np.concatenate([runstarts, [EPC]]))
        odd = (L & 1).astype(bool)
        start2 = np.concatenate([[0], np.cumsum(L + (L & 1))])
        T = int(start2[-1])
        run_of = np.repeat(np.arange(R), L)
        pos2 = start2[:-1][run_of] + (np.arange(EPC) - runstarts[run_of])
        node2 = np.full(T, -1, np.int64)
        h2 = np.zeros((T, D), np.float32)
        node2[pos2] = Xs
        h2[pos2] = Hs
        node2[start2[1:][odd] - 1] = Xs[runstarts[odd]]

        # cut the padded stream at run boundaries into 128 streams
        tgt = np.arange(1, PARTS) * ((T + PARTS - 1) // PARTS)
        ci = np.minimum(np.searchsorted(start2[:-1], tgt), R - 1)
        cuts = np.concatenate([[0], start2[:-1][ci], [T]])
        cnt = np.diff(cuts)
        assert cnt.max() <= SLOTS, f"partition stream overflow: {cnt.max()}"

        node_pad = np.full((PARTS, SLOTS), -1, np.int64)
        h_pad = np.zeros((PARTS, SLOTS, D), np.float32)
        pidx = np.repeat(np.arange(PARTS), cnt)
        eidx = np.arange(T) - np.repeat(cuts[:-1], cnt)
        node_pad[pidx, eidx] = node2
        h_pad[pidx, eidx] = h2
        m = np.zeros((PARTS, SLOTS), np.float32)
        m[:, 1:] = node_pad[:, 1:] == node_pad[:, :-1]
        m2 = m[:, 0::2]                       # pair-level mask [PARTS, 1280]

        # device layout per channel row: [A(1280) | B(1280)]
        dev = np.empty((PARTS, D, SLOTS), np.float32)
        dev[:, :, :HLEN] = h_pad[:, 0::2, :].transpose(0, 2, 1)
        dev[:, :, HLEN:] = h_pad[:, 1::2, :].transpose(0, 2, 1)
        h_dev = dev.reshape(PARTS, FREE).astype(ml_dtypes.bfloat16)
        m_dev = np.ascontiguousarray(m2).astype(ml_dtypes.bfloat16)
        in_maps.append({"h": np.ascontiguousarray(h_dev), "m": m_dev})
        metas.append(node_pad[:, 0::2])       # node id per pair

    _prog_cache["last_inputs"] = in_maps
    # The very first execution of a freshly loaded program has been
    # observed (once) to return corrupted results; correct runs are
    # bit-identical. Run until two consecutive executions agree.
    res = run_bass_kernel_spmd(nc, in_maps, core_ids=list(range(CORES)),
                               trace=False)
    for _ in range(3):
        res2 = run_bass_kernel_spmd(nc, in_maps, core_ids=list(range(CORES)),
                                    trace=False)
        if all(
            np.array_equal(
                res.results[c]["s"].view(np.uint16),
                res2.results[c]["s"].view(np.uint16))
            for c in range(CORES)
        ):
            break
        res = res2

    out = np.zeros((N, D), np.float32)
    for c in range(CORES):
        node_pair = metas[c]                  # [PARTS, 1280]
        s = np.asarray(res.results[c]["s"]).astype(np.float32)
        s = s.reshape(PARTS, D, HLEN)
        nxt = np.concatenate(
            [node_pair[:, 1:], np.full((PARTS, 1), -2, np.int64)], axis=1)
        is_end = (node_pair >= 0) & (node_pair != nxt)
        pp, ii = np.nonzero(is_end)
        nodes = node_pair[pp, ii]
        vals = s[pp, :, ii]
        # within one core each node has exactly one run end -> unique idx
        out[nodes] += vals
    return out


# revision 13
# speedup vs baseline: 1.8226x; 1.0538x over previous
"""Trainium2 Bass kernel for segment_sum (scatter-add of edge features into nodes).

Strategy: 2M edges split contiguously across 8 NeuronCores (250k each).
Host-side prep (layout only, no FP arithmetic): sort each core's edges by
node id, pad every node run to EVEN length (pad slots carry h=0 inside
the run), cut the padded stream into 128 partition streams at run
boundaries, pad each to 2560 slots, and deinterleave each channel row
into step-1 A/B halves (A = even slots, B = odd slots of each pair).

Device (per core):
  1. Pairing pass: A += B with a plain tensor_tensor add (on GPSIMD if
     GPSIMD_TT, else DVE). All operands are bf16 step-1 (DVE would run
     2x mode); each pair is two edges of the SAME node (guaranteed by
     the even-run padding), so A becomes the pair-sum stream at half
     the length.
  2. Segmented scan over A on the DVE:  state = m2[t]*state + A[t]
     (fp32 state, ~2.08 ns/el) per feature channel. At the last pair of
     each node run, state holds that node's complete per-core sum.
Single 2560-slot piece; input DMAs, pairing, scans and output DMAs are
pipelined per 4-channel group. The A halves are DMA'd back (bf16); the
host picks the run-end pairs and adds the 8 per-core partials (same
unshard-add as the original baseline).
"""
import numpy as np
import ml_dtypes

import concourse.bass as bass
import concourse.bacc as bacc
import concourse.mybir as mybir
from concourse import tile
from concourse.bass_utils import run_bass_kernel_spmd

BF16 = mybir.dt.bfloat16
OP = mybir.AluOpType

E = 2_000_000
D = 32
N = 100_000
CORES = 8
EPC = E // CORES            # 250_000
PARTS = 128
SLOTS = 2432                # even-run-padded raw slots per partition (max seen ~2350)
HLEN = SLOTS // 2           # 1280 pairs per channel row
FREE = D * SLOTS
GPSIMD_TT = False           # run the pairing adds on GPSIMD (DVE if False)
# channel groups: DMA-in, pairing-TT, scans and DMA-out are pipelined
# per group; the first groups are small so compute starts early, and
# the final out-DMAs are split fine to shrink the tail
GROUPS = [1, 1, 2, 4, 4, 4, 4, 4, 4, 4]
OUT_SPLIT = {9: [2, 1, 1]}  # group idx -> out-DMA sub-splits


def build_program():
    nc = bacc.Bacc("TRN2", target_bir_lowering=False, debug=False,
                   num_devices=CORES)
    h_in = nc.dram_tensor("h", [PARTS, FREE], BF16, kind="ExternalInput")
    m_in = nc.dram_tensor("m", [PARTS, HLEN], BF16, kind="ExternalInput")
    s_out = nc.dram_tensor("s", [PARTS, D * HLEN], BF16,
                           kind="ExternalOutput")
    tt_engine = nc.gpsimd if GPSIMD_TT else nc.vector

    with tile.TileContext(nc) as tc:
        with tc.tile_pool(name="mask", bufs=1) as mp, \
             tc.tile_pool(name="work", bufs=1) as wp:
            mt = mp.tile([PARTS, HLEN], BF16)
            nc.sync.dma_start(mt[:], m_in[:])
            ht = wp.tile([PARTS, FREE], BF16)
            d0 = 0
            for ng in GROUPS:
                nc.sync.dma_start(
                    ht[:, d0 * SLOTS:(d0 + ng) * SLOTS],
                    h_in[:, d0 * SLOTS:(d0 + ng) * SLOTS])
                d0 += ng
            d = 0
            for gi, ng in enumerate(GROUPS):
                # pairing pass: A[d, j] += B[d, j], in place
                base = d * SLOTS
                a_ap = bass.AP(ht.tensor, base,
                               [[FREE, PARTS], [SLOTS, ng], [1, HLEN]])
                b_ap = bass.AP(ht.tensor, base + HLEN,
                               [[FREE, PARTS], [SLOTS, ng], [1, HLEN]])
                tt_engine.tensor_tensor(a_ap, a_ap, b_ap, OP.add)
                for dd in range(d, d + ng):
                    lo = dd * SLOTS
                    # in-place segmented scan over the A half
                    nc.vector.tensor_tensor_scan(
                        ht[:, lo:lo + HLEN], mt[:],
                        ht[:, lo:lo + HLEN], 0.0, OP.mult, OP.add)
                for so, sn in zip(
                        np.cumsum([0] + OUT_SPLIT.get(gi, [ng]))[:-1],
                        OUT_SPLIT.get(gi, [ng])):
                    src = bass.AP(ht.tensor, (d + int(so)) * SLOTS,
                                  [[FREE, PARTS], [SLOTS, int(sn)],
                                   [1, HLEN]])
                    nc.sync.dma_start(
                        s_out[:, (d + int(so)) * HLEN:
                              (d + int(so) + int(sn)) * HLEN], src)
                d += ng
    nc.compile()
    return nc


_prog_cache = {}


def _get_prog():
    if "nc" not in _prog_cache:
        _prog_cache["nc"] = build_program()
    return _prog_cache["nc"]


def kernel(H, X_node, node_num):
    H = np.ascontiguousarray(np.asarray(H, dtype=np.float32))
    X = np.asarray(X_node).astype(np.int64)
    assert H.shape == (E, D) and X.shape == (E,)
    nc = _get_prog()

    in_maps = []
    metas = []
    for c in range(CORES):
        Xc = X[c * EPC:(c + 1) * EPC]
        Hc = H[c * EPC:(c + 1) * EPC]
        perm = np.argsort(Xc, kind="stable")
        Xs = Xc[perm]
        Hs = Hc[perm]
        # node runs; pad each run to even length (pad slot: h=0, same node)
        runstarts = np.concatenate([[0], np.flatnonzero(np.diff(Xs)) + 1])
        R = len(runstarts)
        L = np.diff(np.concatenate([runstarts, [EPC]]))
        odd = (L & 1).astype(bool)
        start2 = np.concatenate([[0], np.cumsum(L + (L & 1))])
        T = int(start2[-1])
        run_of = np.repeat(np.arange(R), L)
        pos2 = start2[:-1][run_of] + (np.arange(EPC) - runstarts[run_of])
        node2 = np.full(T, -1, np.int64)
        h2 = np.zeros((T, D), np.float32)
        node2[pos2] = Xs
        h2[pos2] = Hs
        node2[start2[1:][odd] - 1] = Xs[runstarts[odd]]

        # cut the padded stream at run boundaries into 128 streams
        tgt = np.arange(1, PARTS) * ((T + PARTS - 1) // PARTS)
        ci = np.minimum(np.searchsorted(start2[:-1], tgt), R - 1)
        cuts = np.concatenate([[0], start2[:-1][ci], [T]])
        cnt = np.diff(cuts)
        assert cnt.max() <= SLOTS, f"partition stream overflow: {cnt.max()}"

        node_pad = np.full((PARTS, SLOTS), -1, np.int64)
        h_pad = np.zeros((PARTS, SLOTS, D), np.float32)
        pidx = np.repeat(np.arange(PARTS), cnt)
        eidx = np.arange(T) - np.repeat(cuts[:-1], cnt)
        node_pad[pidx, eidx] = node2
        h_pad[pidx, eidx] = h2
        m = np.zeros((PARTS, SLOTS), np.float32)
        m[:, 1:] = node_pad[:, 1:] == node_pad[:, :-1]
        m2 = m[:, 0::2]                       # pair-level mask [PARTS, 1280]

        # device layout per channel row: [A(1280) | B(1280)]
        dev = np.empty((PARTS, D, SLOTS), np.float32)
        dev[:, :, :HLEN] = h_pad[:, 0::2, :].transpose(0, 2, 1)
        dev[:, :, HLEN:] = h_pad[:, 1::2, :].transpose(0, 2, 1)
        h_dev = dev.reshape(PARTS, FREE).astype(ml_dtypes.bfloat16)
        m_dev = np.ascontiguousarray(m2).astype(ml_dtypes.bfloat16)
        in_maps.append({"h": np.ascontiguousarray(h_dev), "m": m_dev})
        metas.append(node_pad[:, 0::2])       # node id per pair

    _prog_cache["last_inputs"] = in_maps
    # The very first execution of a freshly loaded program has been
    # observed (once) to return corrupted results; correct runs are
    # bit-identical. Run until two consecutive executions agree.
    res = run_bass_kernel_spmd(nc, in_maps, core_ids=list(range(CORES)),
                               trace=False)
    for _ in range(3):
        res2 = run_bass_kernel_spmd(nc, in_maps, core_ids=list(range(CORES)),
                                    trace=False)
        if all(
            np.array_equal(
                res.results[c]["s"].view(np.uint16),
                res2.results[c]["s"].view(np.uint16))
            for c in range(CORES)
        ):
            break
        res = res2

    out = np.zeros((N, D), np.float32)
    for c in range(CORES):
        node_pair = metas[c]                  # [PARTS, 1280]
        s = np.asarray(res.results[c]["s"]).astype(np.float32)
        s = s.reshape(PARTS, D, HLEN)
        nxt = np.concatenate(
            [node_pair[:, 1:], np.full((PARTS, 1), -2, np.int64)], axis=1)
        is_end = (node_pair >= 0) & (node_pair != nxt)
        pp, ii = np.nonzero(is_end)
        nodes = node_pair[pp, ii]
        vals = s[pp, :, ii]
        # within one core each node has exactly one run end -> unique idx
        out[nodes] += vals
    return out
